# revision 29
# baseline (speedup 1.0000x reference)
"""ISTFT kernel for Trainium2 (8 NeuronCores, SPMD).

Math: out = trim(OLA(hann * irfft(spec)) / window_sum), FFT=2048, HOP=512.

v2 formulation:
- The hann window is folded into the spectrum on the host (pointwise
  time-domain window == 3-tap convolution over frequency k), so the
  device matmul uses the PURE DFT basis.
- Radix-2 decimation in frequency: with pure-DFT columns, sample
  n+1024 flips the sign of odd-k rows and n+512 flips k%4==2 rows
  within the even half.  Per frame, three half/quarter-size products
    Gee (k%4==0, K=512)   Geo (k%4==2, K=512)   Go0/Go1 (k odd, K=1024)
  yield all four 512-sample chunks:
    chunk0 = (Gee+Geo) + Go0      chunk2 = (Gee+Geo) - Go0
    chunk1 = (Gee-Geo) + Go1      chunk3 = (Gee-Geo) - Go1
  This halves the tensor-engine work vs the direct windowed-DFT matmul.
- Transposed layout: q (position within a chunk) lives on PSUM
  partitions and frames stream on the matmul free axis, so the
  overlap-add shifts are free-dim slices (legal on DVE):
    out[q, u] = U0[q,u+3] + U1[q,u+2] + W0[q,u+1] + W1[q,u]
  The [512, 2048] per-core output is transposed back on the host.
- Everything runs in bf16 (inputs + basis) with fp32 PSUM/combines;
  measured rel-err ~2.4e-3 vs the fp64 reference (gate is 2e-2).
- The first/last 512 output samples (window-sum edge) are rescaled on
  the host; the interior window-sum is exactly 1.5 and is folded into
  the basis.
"""

import numpy as np
import ml_dtypes

FFT = 2048
HOP = 512
B, F, NB = 4, 4000, 1025
L = (F - 1) * HOP + FFT  # 2049536 full OLA length
OUT = L - FFT            # 2047488 trimmed output length per batch
U = OUT // HOP           # 3999 output chunks per batch
COLS = 2051              # per-core data frames (2048 chunks + 3 halo)
CPAD = 2176              # padded to 17*128 for whole-tile loads
UO = 2048                # output chunks computed per core
NC_USED = 8
NBLK = 5                 # frame blocks: 4 x 512 + 1 x 16 (3-frame halo)
BLKW = [512, 512, 512, 512, 16]
UWW = 520                # UW tiles: 512 cols + 3 halo cols (padded)
TINY = np.float32(np.finfo(np.float32).tiny)
BF16 = ml_dtypes.bfloat16

# frequency-class row order (after the window fold): EE | EO | O
_kEE_re = np.arange(0, 1025, 4)   # 257
_kEE_im = np.arange(4, 1021, 4)   # 255
_kEO_re = np.arange(2, 1023, 4)   # 256
_kEO_im = np.arange(2, 1023, 4)   # 256
_kO_re = np.arange(1, 1024, 2)    # 512
_kO_im = np.arange(1, 1024, 2)    # 512

_prog_cache = {}
_const_cache = {}


def _hann64(n):
    return 0.5 - 0.5 * np.cos(2.0 * np.pi * np.arange(n) / n)


def _build_constants():
    """de [1024,512] bf16 (D_ee | D_eo), do [1024,1024] bf16
    (D_o cols: n=q | n=512+q), plus window-sum edge fixups e0/e1."""
    if "de" in _const_cache:
        c = _const_cache
        return c["de"], c["do"], c["e0"], c["e1"]
    a = np.full(NB, 2.0)
    a[0] = 1.0
    a[-1] = 1.0
    g = 2.0 / 3.0  # 1/window_sum interior (=1/1.5)

    def crow(kk, n):
        return np.cos(2 * np.pi * np.outer(kk, n) / FFT) * (a[kk][:, None] / FFT) * g

    def srow(kk, n):
        return -np.sin(2 * np.pi * np.outer(kk, n) / FFT) * (a[kk][:, None] / FFT) * g

    q = np.arange(HOP)
    de = np.concatenate(
        [crow(_kEE_re, q), srow(_kEE_im, q),
         crow(_kEO_re, q), srow(_kEO_im, q)], axis=0
    ).astype(BF16)
    do_ = np.concatenate(
        [np.concatenate([crow(_kO_re, q), srow(_kO_im, q)], axis=0),
         np.concatenate([crow(_kO_re, 512 + q), srow(_kO_im, 512 + q)], axis=0)],
        axis=1,
    ).astype(BF16)

    # window_sum edge fixups for the first/last trimmed 512 samples
    w32 = _hann64(FFT).astype(np.float32)
    wsq = np.zeros(L, np.float32)
    idx = (np.arange(F) * HOP)[:, None] + np.arange(FFT)[None, :]
    np.add.at(wsq, idx.ravel(), np.tile(w32 * w32, F))
    ws = np.where(wsq > TINY, wsq, np.float32(1.0))
    half = FFT // 2
    ws_t = ws[half:L - half]
    e0 = (np.float32(1.5) / ws_t[:HOP]).astype(np.float32)
    e1 = (np.float32(1.5) / ws_t[-HOP:]).astype(np.float32)
    _const_cache.update(de=de, do=do_, e0=e0, e1=e1)
    return de, do_, e0, e1


def _conv_spec(re, im):
    """Fold periodic hann into the spectrum: X' = conv_k(X, [-1/4, 1/2, -1/4])
    with Hermitian boundaries (and irfft's implicit Im==0 at DC/Nyquist)."""
    re = re.astype(np.float32)
    im = im.astype(np.float32)
    rp = np.empty_like(re)
    ip = np.zeros_like(im)
    rp[..., 1:-1] = 0.5 * re[..., 1:-1] - 0.25 * (re[..., :-2] + re[..., 2:])
    rp[..., 0] = 0.5 * re[..., 0] - 0.5 * re[..., 1]
    rp[..., -1] = 0.5 * re[..., -1] - 0.5 * re[..., -2]
    ip[..., 2:-2] = 0.5 * im[..., 2:-2] - 0.25 * (im[..., 1:-3] + im[..., 3:-1])
    ip[..., 1] = 0.5 * im[..., 1] - 0.25 * im[..., 2]          # im[0] == 0
    ip[..., -2] = 0.5 * im[..., -2] - 0.25 * im[..., -3]       # im[-1] == 0
    return rp, ip


def _build_program(reps=1):
    import concourse.bacc as bacc
    import concourse.tile as tile
    import concourse.bass as bass

    key = ("v2", reps)
    if key in _prog_cache:
        return _prog_cache[key]
    dt = bass.mybir.dt.float32
    bf = bass.mybir.dt.bfloat16
    act_copy = bass.mybir.ActivationFunctionType.Copy
    nc = bacc.Bacc(None, target_bir_lowering=False, debug=True)
    spec = nc.dram_tensor("spec", [2048, CPAD], bf, kind="ExternalInput")
    de = nc.dram_tensor("de", [1024, 512], bf, kind="ExternalInput")
    do = nc.dram_tensor("do", [1024, 1024], bf, kind="ExternalInput")
    out = nc.dram_tensor("out", [HOP, UO], dt, kind="ExternalOutput")

    with tile.TileContext(nc) as tc:
        with tc.tile_pool(name="const", bufs=2) as constp, \
             tc.tile_pool(name="spec", bufs=3) as specp, \
             tc.tile_pool(name="psum", bufs=2, space="PSUM") as psump, \
             tc.tile_pool(name="ge", bufs=3) as gep, \
             tc.tile_pool(name="uw", bufs=2) as uwp, \
             tc.tile_pool(name="osb", bufs=4) as osbp:
            # Flat (rep, block) pipeline: block loads are issued two items
            # ahead on the sync queue (consts on gpsimd), so the next rep's
            # head never queues behind the previous rep's tail.  The scalar
            # (ACT) queue stays clear for the critical PSUM-drain copies.
            items = [(r, bk) for r in range(reps) for bk in range(NBLK)]
            sp = {}      # (r, bk) -> {t: tile}
            consts = {}  # r -> (de_sb, do_sb)

            def _alloc_consts(r):
                de_sb = constp.tile([128, 8, 512], bf, tag="de")
                do_sb = constp.tile([128, 8, 1024], bf, tag="do")
                consts[r] = (de_sb, do_sb)

            def _const_load(r, t, eng):
                de_sb, do_sb = consts[r]
                if t < 8:
                    eng.dma_start(
                        out=de_sb[:, t, :], in_=de[128 * t:128 * (t + 1), :]
                    )
                else:
                    eng.dma_start(
                        out=do_sb[:, t - 8, :],
                        in_=do[128 * (t - 8):128 * (t - 7), :],
                    )

            def _spec_load(r, bk, t, eng):
                w = BLKW[bk]
                st = specp.tile([128, 512], bf, tag=f"sp{t}")
                eng.dma_start(
                    out=st[:, :w],
                    in_=spec[128 * t:128 * (t + 1), 512 * bk:512 * bk + w],
                )
                sp.setdefault((r, bk), {})[t] = st

            # Cold head: consts + blocks 0-1 of rep 0, interleaved in the
            # order block-0 matmuls consume them (EE: de0-3/sp0-3,
            # EO: de4-7/sp4-7, O: do/sp8-15), alternating sync/gpsimd.
            _alloc_consts(0)
            for t in range(16):
                _const_load(0, t, nc.sync if t % 2 == 0 else nc.gpsimd)
                _spec_load(0, 0, t, nc.gpsimd if t % 2 == 0 else nc.sync)
            for t in range(16):
                _spec_load(0, 1, t, nc.sync if t % 2 == 0 else nc.gpsimd)

            uw_prev = None
            for i, (_rep, bk) in enumerate(items):
                w = BLKW[bk]
                if i + 2 < len(items):
                    nr, nbk = items[i + 2]
                    if nbk == 0:
                        _alloc_consts(nr)
                        for t in range(16):
                            _const_load(nr, t, nc.gpsimd)
                    for t in range(16):
                        _spec_load(nr, nbk, t, nc.sync)
                spb = sp.pop((_rep, bk))
                de_sb, do_sb = consts[_rep]
                if bk == 0:
                    uw_prev = None
                uw_cur = {}
                for s in range(4):
                        q0 = 128 * s
                        gee = psump.tile([128, 512], dt, tag="gee")
                        geo = psump.tile([128, 512], dt, tag="geo")
                        go0 = psump.tile([128, 512], dt, tag="go0")
                        go1 = psump.tile([128, 512], dt, tag="go1")
                        for kt in range(4):
                            nc.tensor.matmul(
                                gee[:, :w],
                                de_sb[:, kt, q0:q0 + 128],
                                spb[kt][:, :w],
                                start=(kt == 0), stop=(kt == 3),
                            )
                        for kt in range(4):
                            nc.tensor.matmul(
                                geo[:, :w],
                                de_sb[:, 4 + kt, q0:q0 + 128],
                                spb[4 + kt][:, :w],
                                start=(kt == 0), stop=(kt == 3),
                            )
                        for kt in range(8):
                            nc.tensor.matmul(
                                go0[:, :w],
                                do_sb[:, kt, q0:q0 + 128],
                                spb[8 + kt][:, :w],
                                start=(kt == 0), stop=(kt == 7),
                            )
                        for kt in range(8):
                            nc.tensor.matmul(
                                go1[:, :w],
                                do_sb[:, kt, 512 + q0:512 + q0 + 128],
                                spb[8 + kt][:, :w],
                                start=(kt == 0), stop=(kt == 7),
                            )
                        # DVE reads at most ONE operand from PSUM per op:
                        # copy Gee to SBUF once, then pair it/derived tiles
                        # with one PSUM operand each.
                        gee_sb = gep.tile([128, 512], dt, tag="gee_sb")
                        ge0 = gep.tile([128, 512], dt, tag="ge0")
                        ge1 = gep.tile([128, 512], dt, tag="ge1")
                        nc.scalar.activation(gee_sb[:, :w], gee[:, :w], act_copy)
                        nc.vector.tensor_add(ge0[:, :w], gee_sb[:, :w], geo[:, :w])
                        nc.vector.tensor_sub(ge1[:, :w], gee_sb[:, :w], geo[:, :w])
                        last = bk == NBLK - 1
                        if not last:
                            # UW tiles carry 3 halo cols (512:515) written by
                            # the NEXT block so assembly is 3 full-width adds
                            u0 = uwp.tile([128, UWW], bf, tag=f"u0_{s}")
                            u1 = uwp.tile([128, UWW], bf, tag=f"u1_{s}")
                            w0 = uwp.tile([128, UWW], bf, tag=f"w0_{s}")
                            w1 = uwp.tile([128, UWW], bf, tag=f"w1_{s}")
                            nc.vector.tensor_add(
                                u0[:, :w], ge0[:, :w], go0[:, :w])
                            nc.vector.tensor_sub(
                                w0[:, :w], ge0[:, :w], go0[:, :w])
                            nc.vector.tensor_add(
                                u1[:, :w], ge1[:, :w], go1[:, :w])
                            nc.vector.tensor_sub(
                                w1[:, :w], ge1[:, :w], go1[:, :w])
                            uw_cur[s] = (u0, u1, w0, w1)
                        if bk >= 1:
                            u0p, u1p, w0p, w1p = uw_prev[s]
                            nc.vector.tensor_add(
                                u0p[:, 512:515], ge0[:, 0:3], go0[:, 0:3])
                            nc.vector.tensor_add(
                                u1p[:, 512:514], ge1[:, 0:2], go1[:, 0:2])
                            nc.vector.tensor_sub(
                                w0p[:, 512:513], ge0[:, 0:1], go0[:, 0:1])
                            t1 = osbp.tile([128, 512], dt, tag="t1")
                            t2 = osbp.tile([128, 512], dt, tag="t2")
                            ob = osbp.tile([128, 512], dt, tag="ob")
                            nc.gpsimd.tensor_add(
                                t1[:, :], u0p[:, 3:515], u1p[:, 2:514])
                            nc.gpsimd.tensor_add(
                                t2[:, :], w0p[:, 1:513], w1p[:, 0:512])
                            nc.gpsimd.tensor_add(ob[:, :], t1[:, :], t2[:, :])
                            nc.gpsimd.dma_start(
                                out=out[128 * s:128 * (s + 1),
                                        512 * (bk - 1):512 * bk],
                                in_=ob[:, :],
                            )
                uw_prev = uw_cur
    nc.compile()
    _prog_cache[key] = nc
    return nc


def _class_rows(re, im):
    """Fused conv+gather: class-ordered convolved rows [..., 2048] using
    strided slices only (no fancy indexing).  Matches
    concat(conv(re)[kEE_re], conv(im)[kEE_im], ..., axis=-1)."""
    out = np.empty(re.shape[:-1] + (2048,), np.float32)
    # EE re: k=0,4..1024 (257); boundaries re[-1]=re[1], re[1025]=re[1023]
    o = out[..., 0:257]
    np.multiply(re[..., 0::4], 0.5, out=o)
    o[..., 0] -= 0.25 * re[..., 1]        # reflected k-1 term (re[-1]=re[1])
    o[..., 1:] -= 0.25 * re[..., 3:1024:4]
    o[..., :-1] -= 0.25 * re[..., 1:1022:4]
    o[..., -1] -= 0.25 * re[..., 1023]
    # EE im: k=4..1020 (255); all interior
    o = out[..., 257:512]
    np.multiply(im[..., 4:1021:4], 0.5, out=o)
    o -= 0.25 * im[..., 3:1020:4]
    o -= 0.25 * im[..., 5:1022:4]
    # EO re: k=2..1022 (256)
    o = out[..., 512:768]
    np.multiply(re[..., 2:1023:4], 0.5, out=o)
    o -= 0.25 * re[..., 1:1022:4]
    o -= 0.25 * re[..., 3:1024:4]
    # EO im: k=2..1022 (256)
    o = out[..., 768:1024]
    np.multiply(im[..., 2:1023:4], 0.5, out=o)
    o -= 0.25 * im[..., 1:1022:4]
    o -= 0.25 * im[..., 3:1024:4]
    # O re: k=1,3..1023 (512)
    o = out[..., 1024:1536]
    np.multiply(re[..., 1::2], 0.5, out=o)
    o -= 0.25 * re[..., 0:1024:2]
    o -= 0.25 * re[..., 2::2]
    # O im: k=1,3..1023 (512); im[0] and im[1024] count as zero
    o = out[..., 1536:2048]
    np.multiply(im[..., 1::2], 0.5, out=o)
    o[..., 1:] -= 0.25 * im[..., 2:1023:2]
    o[..., :-1] -= 0.25 * im[..., 2:1023:2]
    return out


def _stage_inputs(spec_real, spec_imag):
    """Per-core bf16 [2048, CPAD] slices: class-ordered convolved spectrum
    rows x padded local frame columns."""
    X = _class_rows(np.asarray(spec_real, np.float32),
                    np.asarray(spec_imag, np.float32))     # [B, F, 2048] f32
    Xb = X.astype(BF16)                                    # halve bytes early
    slices = []
    for c in range(NC_USED):
        b, h = c // 2, c % 2
        sl = np.zeros((2048, CPAD), BF16)
        # frame columns map to padded frames [h*2000, h*2000+2051); padded
        # frame 1..F -> spec frame (padded - 1)
        lo, hi = h * 2000, h * 2000 + COLS
        dlo, dhi = max(lo, 1), min(hi, F + 1)
        sl[:, dlo - lo:dhi - lo] = Xb[b, dlo - 1:dhi - 1].T
        slices.append(sl)
    return slices


def _make_bench_in_maps(rng):
    """Random-input in_maps with the right shapes/dtypes (for timing)."""
    de, do_, _, _ = _build_constants()
    return [
        {"spec": rng.standard_normal((2048, CPAD), dtype=np.float32).astype(BF16),
         "de": de, "do": do_}
        for _ in range(NC_USED)
    ]


def _run(in_maps, trace=False):
    from concourse.bass_utils import run_bass_kernel_spmd
    nc = _build_program()
    return run_bass_kernel_spmd(nc, in_maps, list(range(NC_USED)), trace=trace)


def kernel(spec_real, spec_imag, _trace=False, _ret_raw=False):
    spec_real = np.ascontiguousarray(spec_real, dtype=np.float32)
    spec_imag = np.ascontiguousarray(spec_imag, dtype=np.float32)
    de, do_, e0, e1 = _build_constants()
    slices = _stage_inputs(spec_real, spec_imag)
    in_maps = [{"spec": sl, "de": de, "do": do_} for sl in slices]

    res = _run(in_maps, trace=_trace)

    chunks = np.empty((B, U, HOP), np.float32)
    for b in range(B):
        o0 = np.asarray(res.results[2 * b]["out"], np.float32).T      # [2048, 512]
        o1 = np.asarray(res.results[2 * b + 1]["out"], np.float32).T
        chunks[b, :2000] = o0[:2000]
        chunks[b, 2000:] = o1[:U - 2000]
    y = chunks.reshape(B, OUT)
    y[:, :HOP] *= e0
    y[:, -HOP:] *= e1
    if _ret_raw:
        return y, res
    return y
